# revision 8
# baseline (speedup 1.0000x reference)
"""Ribbon MoE kernel: data-parallel over tokens with chain-ordered classes.

Each core gets ~2048 whole tokens (x loaded ONCE, y written ONCE — the
baseline expert-parallel kernel paid 2x on both). Tokens are ordered along
an Eulerian circuit of the expert-pair multigraph so that every expert's
tokens form contiguous ranges; each core's arc is a chain of class pieces.

Per-core program (compiled per core — ranges are core-specific constants):
  mm1  h[r, t] (PSUM) = sum_kc Wa[slot]^T @ xT       per expert occurrence
  scale h_sb = h * combine-weight (broadcast via 1-partition ones matmul)
  mm2  yT[o, t] (PSUM) accumulates BOTH experts of each token, per o-chunk
  out  yT -> DRAM bf16; host transposes and scatters.
"""

import os
import sys
from contextlib import ExitStack

import numpy as np

if os.environ.get("JAX_PLATFORMS", None) == "cpu" and "jax" not in sys.modules:
    os.environ.pop("JAX_PLATFORMS")

for _p in ("/opt/trn_rl_repo",):
    if _p not in sys.path and os.path.isdir(_p):
        sys.path.insert(0, _p)

import ml_dtypes  # noqa: E402

import concourse.tile as tile  # noqa: E402
from concourse import bacc, mybir  # noqa: E402

BF16 = mybir.dt.bfloat16
NP_BF16 = ml_dtypes.bfloat16
F32 = mybir.dt.float32

N_EXPERTS = 8
D = 2048
R = 128
O = 2048
KC = D // 128
TB = 512

_PROGRAM_CACHE: dict[tuple, object] = {}
LAST_RUN = {"exec_time_ns": None, "mean_exec_time_ns": None, "per_core_ns": None}


# ---------------------------------------------------------------- host: ribbon
def _route(x, router_w):
    norm = np.maximum(np.sqrt(np.einsum("td,td->t", x, x, dtype=np.float64)), 1e-12)
    logits = (x @ router_w) / norm[:, None].astype(np.float32)
    m = logits.max(-1, keepdims=True)
    p = np.exp(logits - m, dtype=np.float32)
    p /= p.sum(-1, keepdims=True)
    t_idx = np.arange(x.shape[0])
    e1 = p.argmax(-1)
    w1 = p[t_idx, e1]
    p2 = p.copy()
    p2[t_idx, e1] = -np.inf
    e2 = p2.argmax(-1)
    w2 = p[t_idx, e2]
    s = w1 + w2
    return e1, e2, (w1 / s).astype(np.float32), (w2 / s).astype(np.float32)


def _eulerian_circuit(edges, rng):
    adj = {}
    for i, (a, b) in enumerate(edges):
        adj.setdefault(a, []).append((b, i))
        adj.setdefault(b, []).append((a, i))
    if rng is not None:
        for v in adj:
            rng.shuffle(adj[v])
    used = [False] * len(edges)
    stack = [edges[0][0]]
    path = []
    while stack:
        v = stack[-1]
        found = False
        while adj[v]:
            u, ei = adj[v].pop()
            if not used[ei]:
                used[ei] = True
                stack.append(u)
                found = True
                break
        if not found:
            path.append(stack.pop())
    if not all(used):
        raise RuntimeError("not connected")
    return path[::-1]


_MATCHINGS = [
    [(0, 1), (2, 3), (4, 5), (6, 7)],
    [(0, 2), (1, 3), (4, 6), (5, 7)],
    [(0, 3), (1, 2), (4, 7), (5, 6)],
    [(0, 4), (1, 5), (2, 6), (3, 7)],
    [(0, 5), (1, 4), (2, 7), (3, 6)],
    [(0, 6), (1, 7), (2, 4), (3, 5)],
    [(0, 7), (1, 6), (2, 5), (3, 4)],
]


def _build_circuit(class_tokens, matching, seed):
    edges = sorted(class_tokens.keys())
    dup = [tuple(sorted(m)) for m in matching if tuple(sorted(m)) in class_tokens]
    deg = np.zeros(N_EXPERTS, int)
    for a, b in edges + dup:
        deg[a] += 1
        deg[b] += 1
    odd = [v for v in range(N_EXPERTS) if deg[v] % 2]
    while odd:
        a = odd.pop()
        for b in odd:
            if (min(a, b), max(a, b)) in class_tokens:
                dup.append((min(a, b), max(a, b)))
                odd.remove(b)
                break
        else:
            raise RuntimeError("cannot fix parity")
    rng = np.random.default_rng(seed)
    circuit = _eulerian_circuit(edges + dup, rng)
    inst = [(circuit[i], circuit[i + 1]) for i in range(len(circuit) - 1)]
    n_inst_of = {}
    for va, vb in inst:
        key = (min(va, vb), max(va, vb))
        n_inst_of[key] = n_inst_of.get(key, 0) + 1
    inst_counts = []
    seen = {}
    for va, vb in inst:
        key = (min(va, vb), max(va, vb))
        k, n = n_inst_of[key], class_tokens[key].size
        j = seen.get(key, 0)
        seen[key] = j + 1
        inst_counts.append(n // k + (1 if j < n % k else 0))
    return inst, inst_counts


def _cut_arcs(inst, inst_counts, cuts):
    bounds = np.concatenate([[0], np.cumsum(inst_counts)])
    arcs = []
    for c in range(len(cuts) - 1):
        lo, hi = cuts[c], cuts[c + 1]
        pieces = []
        for i, (va, vb) in enumerate(inst):
            a, b = max(bounds[i], lo), min(bounds[i + 1], hi)
            if a < b:
                pieces.append((va, vb, int(b - a)))
        arcs.append(pieces)
    return arcs


def _arc_cost(pieces):
    T = sum(p[2] for p in pieces)
    verts = [pieces[0][0]] + [p[1] for p in pieces]
    ns = len(set(verts))
    dma = (T * D * 2 * 2 + ns * 2 * 128 * 2048 * 2 + T * 2 * 2 * 2) / 360.0
    pe = (2 * T * 32 + 2 * T) / 2.4
    return max(dma, pe) + 3500.0


def _balance_cuts(inst, inst_counts, n_cores=8, iters=200):
    T = int(np.sum(inst_counts))
    cuts = [round(T * i / n_cores) for i in range(n_cores + 1)]
    best = None
    for _ in range(iters):
        arcs = _cut_arcs(inst, inst_counts, cuts)
        costs = [_arc_cost(a) for a in arcs]
        worst = max(costs)
        if best is None or worst < best[0]:
            best = (worst, list(cuts))
        w = int(np.argmax(costs))
        moved = False
        for delta in (64, 32, 16, 8):
            if w > 0 and costs[w - 1] < worst:
                nc_ = list(cuts)
                nc_[w] += delta
                if nc_[w] < nc_[w + 1]:
                    cuts, moved = nc_, True
                    break
            if w < n_cores - 1 and costs[w + 1] < worst:
                nc_ = list(cuts)
                nc_[w + 1] -= delta
                if nc_[w + 1] > nc_[w]:
                    cuts, moved = nc_, True
                    break
        if not moved:
            break
    return best[1]


def _plan_ribbon(e1, e2, n_cores=8, n_trials=120):
    """-> (class_tokens, inst, inst_counts, cuts) minimizing max core cost."""
    lo = np.minimum(e1, e2)
    hi = np.maximum(e1, e2)
    cid = lo * N_EXPERTS + hi
    class_tokens = {}
    for a in range(N_EXPERTS):
        for b in range(a + 1, N_EXPERTS):
            idx = np.nonzero(cid == a * N_EXPERTS + b)[0]
            if idx.size:
                class_tokens[(a, b)] = idx
    best = None
    for trial in range(n_trials):
        matching = _MATCHINGS[trial % len(_MATCHINGS)]
        try:
            inst0, counts0 = _build_circuit(class_tokens, matching, trial)
        except RuntimeError:
            continue
        # the circuit is closed, so any rotation of the instance list is
        # also a valid chain ordering; rotations move the cut boundaries
        # relative to the classes (different NS distributions)
        for rot in range(0, len(inst0), 4):
            inst = inst0[rot:] + inst0[:rot]
            inst_counts = counts0[rot:] + counts0[:rot]
            cuts = _balance_cuts(inst, inst_counts, n_cores)
            arcs = _cut_arcs(inst, inst_counts, cuts)
            cost = max(_arc_cost(a) for a in arcs)
            if best is None or cost < best[0]:
                best = (cost, inst, inst_counts, cuts)
    _, inst, inst_counts, cuts = best
    return class_tokens, inst, inst_counts, cuts


# ------------------------------------------------------------- device program
def _arc_meta(pieces):
    """Static per-core structure used both for program build and host arrays.

    Returns dict with:
      T, K, bounds b[0..K], verts v[0..K], slots (distinct experts in first-
      appearance order), slot_of[j] for each occurrence j, occ ranges
      (s_j, e_j), h offsets o_j, H.
    """
    K = len(pieces)
    lens = [p[2] for p in pieces]
    bounds = [0]
    for L in lens:
        bounds.append(bounds[-1] + L)
    T = bounds[-1]
    verts = [pieces[0][0]] + [p[1] for p in pieces]
    slots = []
    slot_of = []
    for v in verts:
        if v not in slots:
            slots.append(v)
        slot_of.append(slots.index(v))
    occ = []
    for j in range(K + 1):
        s = bounds[max(j - 1, 0)]
        e = bounds[min(j + 1, K)]
        occ.append((s, e))
    offs = [0]
    for s, e in occ:
        offs.append(offs[-1] + (e - s))
    H = offs[-1]
    return dict(
        T=T, K=K, bounds=bounds, verts=verts, slots=slots, slot_of=slot_of,
        occ=occ, offs=offs[:-1], H=H, NS=len(slots), variant=0,
    )


def _prog_key(meta):
    return (
        meta["T"], tuple(meta["bounds"]), tuple(meta["slot_of"]), meta["NS"],
        meta.get("variant", 0),
    )


def _build_program(meta):
    T, K, bounds = meta["T"], meta["K"], meta["bounds"]
    occ, offs, H, NS = meta["occ"], meta["offs"], meta["H"], meta["NS"]
    slot_of = meta["slot_of"]
    variant = meta.get("variant", 0)
    v_wb0_first = bool(variant & 1)  # need-0 wb slots before x1 in the head
    v_mm20_first = not bool(variant & 2)  # mm2(0) before mm1(1)
    v_two_small = bool(variant & 4)  # two small head blocks
    v_small_tail2 = bool(variant & 8)  # two small tail blocks
    n_warm = 6 if variant & 16 else 10  # PE p-state warmup matmuls
    v_x0b_first = bool(variant & 32)  # x0b before wa0b/cv in the head

    nc = bacc.Bacc("TRN2", target_bir_lowering=False, debug=False, num_devices=1)
    xT = nc.dram_tensor("xT", [D, T], BF16, kind="ExternalInput").ap()
    wa = nc.dram_tensor("wa", [NS, 128, KC * R], BF16, kind="ExternalInput").ap()
    wb = nc.dram_tensor("wb", [NS, 128, O], BF16, kind="ExternalInput").ap()
    cv = nc.dram_tensor("cv", [1, H], BF16, kind="ExternalInput").ap()
    yT = nc.dram_tensor("yT", [O, T], BF16, kind="ExternalOutput").ap()

    xTr = xT.rearrange("(kc p) t -> p kc t", p=128)
    yTr = yT.rearrange("(g p) t -> p g t", p=128)
    war = wa.rearrange("s p f -> p s f")
    wbr = wb.rearrange("s p f -> p s f")
    # block sizes: small head (fast pipeline prime) and tail (fast drain),
    # balanced middle; every size ≥ 256 tokens keeps DMA lines ≥ 512B.
    if T <= 1024:
        n_blk = (T + TB - 1) // TB
        bsz = [T // n_blk + (1 if i < T % n_blk else 0) for i in range(n_blk)]
    elif v_two_small and v_small_tail2 and T > 1536:
        mid = T - 1024
        nmid = (mid + TB - 1) // TB
        bsz = (
            [256, 256]
            + [mid // nmid + (1 if i < mid % nmid else 0) for i in range(nmid)]
            + [256, 256]
        )
    elif v_two_small and T > 1280:
        mid = T - 768
        nmid = (mid + TB - 1) // TB
        bsz = (
            [256, 256]
            + [mid // nmid + (1 if i < mid % nmid else 0) for i in range(nmid)]
            + [256]
        )
    elif v_small_tail2 and T > 1280:
        mid = T - 768
        nmid = (mid + TB - 1) // TB
        bsz = (
            [256]
            + [mid // nmid + (1 if i < mid % nmid else 0) for i in range(nmid)]
            + [256, 256]
        )
    else:
        mid = T - 512
        nmid = (mid + TB - 1) // TB
        bsz = (
            [256]
            + [mid // nmid + (1 if i < mid % nmid else 0) for i in range(nmid)]
            + [256]
        )
    n_blk = len(bsz)
    bstart = [0]
    for s in bsz:
        bstart.append(bstart[-1] + s)

    def blk(b):
        return bstart[b], bstart[b + 1]

    # first block needing each weight slot (slots are numbered in chain
    # first-appearance order, so first use of slot s is occ j with
    # slot_of[j] == s, covering tokens from occ[j][0])
    first_tok = {}
    for j in range(K + 1):
        s = slot_of[j]
        if s not in first_tok:
            first_tok[s] = occ[j][0]

    def tok_to_blk(t):
        for b in range(n_blk):
            if t < bstart[b + 1]:
                return b
        return n_blk - 1

    need_blk = {s: tok_to_blk(first_tok[s]) for s in range(NS)}

    with tile.TileContext(nc) as tc, ExitStack() as ctx:
        wpool = ctx.enter_context(tc.tile_pool(name="w", bufs=1))
        xpool = ctx.enter_context(tc.tile_pool(name="x", bufs=3))
        hpool = ctx.enter_context(tc.tile_pool(name="h", bufs=1))
        ypool = ctx.enter_context(tc.tile_pool(name="y", bufs=6))
        hps = ctx.enter_context(tc.tile_pool(name="hps", bufs=2, space="PSUM"))
        yps = ctx.enter_context(tc.tile_pool(name="yps", bufs=3, space="PSUM"))

        # ---- head: cv (tiny) first, then wa slot 0 + x block 0 so mm1(0)
        # starts early; remaining weight slots are loaded just-in-time in
        # chain order, interleaved with x blocks. While the head DMAs land,
        # PE broadcasts the combine weights (dummy+cb matmuls double as
        # p-state warmup); Pool does the PSUM->SBUF cb copies (idle early).
        ones_sb = wpool.tile([1, TB], BF16)
        nc.gpsimd.memset(ones_sb[:], 1.0)
        cv_sb = wpool.tile([1, H], BF16)
        wa_sb = wpool.tile([128, NS, KC * R], BF16)
        wb_sb = wpool.tile([128, NS, O], BF16)
        cb_sb = wpool.tile([128, H], BF16)
        loaded = set()
        loaded_b = set()

        def load_wa(s):
            if s in loaded:
                return
            loaded.add(s)
            nc.sync.dma_start(wa_sb[:, s, :], war[:, s, :])

        def load_wb(s):
            if s in loaded_b:
                return
            loaded_b.add(s)
            nc.sync.dma_start(wb_sb[:, s, :], wbr[:, s, :])

        xts = {}
        def load_x_half(b, half):
            t0, t1 = blk(b)
            tb = t1 - t0
            if half == 0:
                xt = xpool.tile([128, KC, TB], BF16, tag="xt")
                xts[b] = xt
            q = KC // 2
            nc.sync.dma_start(
                xts[b][:, half * q : (half + 1) * q, :tb],
                xTr[:, half * q : (half + 1) * q, t0:t1],
            )

        def jit_wa(lim):
            for s in range(NS):
                if need_blk[s] <= lim:
                    load_wa(s)

        def jit_wb(lim):
            for s in range(NS):
                if need_blk[s] <= lim:
                    load_wb(s)

        nc.sync.dma_start(wa_sb[:, 0, : KC * R // 2], war[:, 0, : KC * R // 2])
        loaded.add(0)
        load_x_half(0, 0)
        if v_x0b_first:
            load_x_half(0, 1)
            nc.sync.dma_start(
                wa_sb[:, 0, KC * R // 2 :], war[:, 0, KC * R // 2 :]
            )
            nc.sync.dma_start(cv_sb[:], cv[:])
        else:
            nc.sync.dma_start(cv_sb[:], cv[:])
            nc.sync.dma_start(
                wa_sb[:, 0, KC * R // 2 :], war[:, 0, KC * R // 2 :]
            )
            load_x_half(0, 1)
        jit_wa(1)
        head_wb = (
            [s for s in range(NS) if need_blk[s] == 0] if v_wb0_first else [0]
        )
        for s in head_wb:  # first halves: unblock mm2(0) o-chunks 0-7
            loaded_b.add(s)
            nc.sync.dma_start(wb_sb[:, s, : O // 2], wbr[:, s, : O // 2])
        if n_blk > 1:
            load_x_half(1, 0)
        for s in head_wb:
            nc.sync.dma_start(wb_sb[:, s, O // 2 :], wbr[:, s, O // 2 :])
        if n_blk > 1:
            load_x_half(1, 1)
        jit_wb(1)

        # PE warmup: dummy matmuls on the memset ones tile (no DMA dep) keep
        # PE continuously busy from ~0.4us so the p-state ramp completes by
        # the time mm1(0)'s inputs land.
        for _ in range(n_warm):
            warm_ps = yps.tile([128, 2, TB], F32, tag="yp")
            nc.tensor.matmul(
                warm_ps[:, 0, :TB], ones_sb[:, :128], ones_sb[:, :],
                start=True, stop=True,
            )
        # broadcast combine weights to 128 partitions on Pool (idle early,
        # SBUF->SBUF): first the columns blocks 0-1 read, then the rest.
        lim = bstart[min(2, n_blk)]
        cb_head = max(
            offs[j] + (min(occ[j][1], lim) - occ[j][0])
            for j in range(K + 1)
            if occ[j][0] < lim
        )
        nc.gpsimd.partition_broadcast(
            cb_sb[:, :cb_head], cv_sb[0:1, :cb_head]
        )
        if cb_head < H:
            nc.gpsimd.partition_broadcast(
                cb_sb[:, cb_head:], cv_sb[0:1, cb_head:]
            )

        h_sb = hpool.tile([128, H], BF16)

        def mm1_block(b):
            t0, t1 = blk(b)
            for j in range(K + 1):
                s_j, e_j = occ[j]
                ss, se = max(s_j, t0), min(e_j, t1)
                if ss >= se:
                    continue
                seg = se - ss
                ho = offs[j] + (ss - s_j)
                hp = hps.tile([128, TB], F32, tag="hp")
                for kc in range(KC):
                    nc.tensor.matmul(
                        hp[:, :seg],
                        wa_sb[:, slot_of[j], kc * R : (kc + 1) * R],
                        xts[b][:, kc, ss - t0 : se - t0],
                        start=(kc == 0),
                        stop=(kc == KC - 1),
                    )
                nc.vector.tensor_tensor(
                    h_sb[:, ho : ho + seg],
                    hp[:, :seg],
                    cb_sb[:, ho : ho + seg],
                    mybir.AluOpType.mult,
                )

        copy_engines = [nc.vector, nc.scalar, nc.gpsimd]

        deferred_y = []

        def flush_y(upto_b):
            # early-block y stores issue from the DVE queue, positioned after
            # later blocks' h-scales, so their DMA requests don't compete
            # with x/weight loads during the input-heavy phase
            while deferred_y and deferred_y[0][0] <= upto_b:
                _, ys_t, hf_, t0_, t1_, tb_ = deferred_y.pop(0)
                nc.scalar.dma_start(
                    yTr[:, hf_ * 8 : (hf_ + 1) * 8, t0_:t1_], ys_t[:, :, :tb_]
                )

        def mm2_block(b):
            t0, t1 = blk(b)
            tb = t1 - t0
            for hf in range(2):  # 8 o-chunks per half; separate y store each
                ys = ypool.tile([128, 8, TB], BF16, tag="ys")
                for pair in range(4):
                    yp = yps.tile([128, 2, TB], F32, tag="yp")
                    for u in range(2):
                        oc = hf * 8 + pair * 2 + u
                        for i in range(K):
                            ps = max(bounds[i], t0)
                            pe = min(bounds[i + 1], t1)
                            if ps >= pe:
                                continue
                            for j, start in ((i, True), (i + 1, False)):
                                s_j, _ = occ[j]
                                ho = offs[j] + (ps - s_j)
                                nc.tensor.matmul(
                                    yp[:, u, ps - t0 : pe - t0],
                                    wb_sb[:, slot_of[j], oc * 128 : (oc + 1) * 128],
                                    h_sb[:, ho : ho + (pe - ps)],
                                    start=start,
                                    stop=not start,
                                )
                    dst = ys[:, pair * 2 : pair * 2 + 2, :tb]
                    if (hf * 4 + pair) % 2 == 0:
                        nc.vector.tensor_copy(dst, yp[:, :, :tb])
                    else:
                        nc.scalar.copy(dst, yp[:, :, :tb])
                    if b == n_blk - 1:
                        # drain: store each pair via HWDGE as soon as its
                        # copy lands (no SWDGE fixed cost, short tail chain)
                        oc0 = hf * 8 + pair * 2
                        nc.sync.dma_start(
                            yTr[:, oc0 : oc0 + 2, t0:t1],
                            ys[:, pair * 2 : pair * 2 + 2, :tb],
                        )
                if b < n_blk - 3:
                    deferred_y.append((b, ys, hf, t0, t1, tb))
                elif b != n_blk - 1:
                    nc.gpsimd.dma_start(
                        yTr[:, hf * 8 : (hf + 1) * 8, t0:t1], ys[:, :, :tb]
                    )

        # software pipeline: x loads 2 blocks ahead; mm1(b+1) before mm2(b)
        # software pipeline; block 0 computes mm2 before mm1(1) (x1 lands
        # late during the head burst, and mm2(0)'s inputs are ready first)
        mm1_block(0)
        for b in range(n_blk):
            if b + 2 < n_blk:
                load_x_half(b + 2, 0)
                jit_wa(b + 3)
                load_x_half(b + 2, 1)
                jit_wb(b + 2)
            if b == 0 and v_mm20_first:
                mm2_block(0)
                if n_blk > 1:
                    mm1_block(1)
                continue
            if b + 1 < n_blk:
                mm1_block(b + 1)
            flush_y(b - 2)
            mm2_block(b)
        flush_y(n_blk)
        assert not deferred_y
        assert len(loaded) == NS and len(loaded_b) == NS, (loaded, loaded_b, need_blk)

    nc.compile()
    return nc


def _get_program(meta):
    key = _prog_key(meta)
    if key not in _PROGRAM_CACHE:
        _PROGRAM_CACHE[key] = _build_program(meta)
    return _PROGRAM_CACHE[key]


def _sim_ns(meta):
    from concourse.timeline_sim import TimelineSim

    key = ("sim",) + _prog_key(meta)
    if key not in _PROGRAM_CACHE:
        _PROGRAM_CACHE[key] = int(TimelineSim(_get_program(meta)).simulate())
    return _PROGRAM_CACHE[key]


_VARIANT_SET = [0, 1, 3, 4, 5, 7, 8, 12, 16, 17, 20, 21, 23, 28,
                32, 33, 36, 39, 40, 44]


def _pick_variants(arcs, variants=None):
    """Per-core schedule variant chosen by TimelineSim (per-core programs
    make heterogeneous schedules free)."""
    if variants is None:
        variants = _VARIANT_SET
    metas = []
    for pieces in arcs:
        best = None
        for v in variants:
            m = _arc_meta(pieces)
            m["variant"] = v
            t = _sim_ns(m)
            if best is None or t < best[0]:
                best = (t, m)
        metas.append(best[1])
    return metas


def _refine_cuts(inst, inst_counts, cuts, iters=6, tol_ns=400):
    """Move cut positions between adjacent cores based on actual TimelineSim
    costs of the compiled per-core programs (the static cost model misses
    schedule overheads that differ per core)."""
    T = int(np.sum(inst_counts))
    n_cores = len(cuts) - 1
    best_cuts, best_max = list(cuts), None
    for _ in range(iters):
        arcs = _cut_arcs(inst, inst_counts, cuts)
        times = [_sim_ns(m) for m in _pick_variants(arcs)]
        worst = max(times)
        if best_max is None or worst < best_max:
            best_max, best_cuts = worst, list(cuts)
        order = sorted(range(n_cores), key=lambda i: -times[i])
        moved = False
        for w in order[:3]:
            if times[w] < worst - tol_ns:
                break
            # shift tokens from w toward its cheaper neighbor
            for nb, edge in ((w - 1, w), (w + 1, w + 1)):
                if 0 <= nb < n_cores and times[nb] < times[w] - tol_ns:
                    delta = min(96, max(16, (times[w] - times[nb]) // 90))
                    nc_ = list(cuts)
                    if nb < w:
                        nc_[edge] += delta
                    else:
                        nc_[edge] -= delta
                    if nc_[edge] - nc_[edge - 1] >= 256 and nc_[edge + 1] - nc_[edge] >= 256:
                        cuts, moved = nc_, True
                        break
            if moved:
                break
        if not moved:
            break
    return best_cuts


# ------------------------------------------------------------------ execution
def kernel(hidden_states, router_w, Wa, Wb):
    B, S, _ = hidden_states.shape
    x = np.ascontiguousarray(
        np.asarray(hidden_states, dtype=np.float32).reshape(-1, D)
    )
    T_all = x.shape[0]
    router_w = np.asarray(router_w, dtype=np.float32)
    Wa = np.asarray(Wa, dtype=np.float32)
    Wb = np.asarray(Wb, dtype=np.float32)

    e1, e2, c1, c2 = _route(x, router_w)
    cmat = np.zeros((T_all, N_EXPERTS), np.float32)
    cmat[np.arange(T_all), e1] = c1
    cmat[np.arange(T_all), e2] = c2

    class_tokens, inst, inst_counts, cuts0 = _plan_ribbon(e1, e2)
    best = None
    for it, tol in ((16, 300), (12, 150)):
        c2 = _refine_cuts(inst, inst_counts, cuts0, iters=it, tol_ns=tol)
        t2 = max(
            _sim_ns(m) for m in _pick_variants(_cut_arcs(inst, inst_counts, c2))
        )
        if best is None or t2 < best[0]:
            best = (t2, c2)
    cuts = best[1]
    arcs = _cut_arcs(inst, inst_counts, cuts)

    # global token order: per class, tokens consumed across its instances
    consumed = {k: 0 for k in class_tokens}
    order_parts = []
    for (va, vb), cnt in zip(inst, inst_counts):
        key = (min(va, vb), max(va, vb))
        s = consumed[key]
        order_parts.append(class_tokens[key][s : s + cnt])
        consumed[key] = s + cnt
    order = np.concatenate(order_parts)
    assert order.size == T_all

    x_bf = x.astype(NP_BF16)

    metas0 = _pick_variants(arcs)
    metas, in_maps, tok_lists = [], [], []
    for c, pieces in enumerate(arcs):
        meta = metas0[c]
        toks = order[cuts[c] : cuts[c + 1]]
        assert toks.size == meta["T"]
        xTc = np.ascontiguousarray(x_bf[toks].T)
        wa_arr = np.stack(
            [
                np.ascontiguousarray(
                    Wa[v].reshape(KC, 128, R).transpose(1, 0, 2).reshape(128, KC * R)
                ).astype(NP_BF16)
                for v in meta["slots"]
            ]
        )
        wb_arr = np.stack([Wb[v].astype(NP_BF16) for v in meta["slots"]])
        cv_arr = np.zeros((1, meta["H"]), np.float32)
        for j in range(meta["K"] + 1):
            s_j, e_j = meta["occ"][j]
            seg_toks = toks[s_j:e_j]
            cv_arr[0, meta["offs"][j] : meta["offs"][j] + (e_j - s_j)] = cmat[
                seg_toks, meta["verts"][j]
            ]
        metas.append(meta)
        tok_lists.append(toks)
        in_maps.append(
            {"xT": xTc, "wa": wa_arr, "wb": wb_arr, "cv": cv_arr.astype(NP_BF16)}
        )

    ncs = [_get_program(m) for m in metas]
    LAST_RUN["metas"] = metas

    import jax
    from concourse import bass2jax

    devices = jax.devices()
    results = []
    for c in range(len(ncs)):
        for attempt in range(3):
            try:
                with jax.default_device(devices[c % len(devices)]):
                    res = bass2jax.run_bass_via_pjrt(
                        ncs[c], [in_maps[c]], n_cores=1
                    )
                break
            except Exception:  # transient NRT_EXEC_UNIT_UNRECOVERABLE etc.
                if attempt == 2:
                    raise
                try:
                    import jax.extend.backend

                    jax.extend.backend.clear_backends()
                except Exception:
                    pass
                import time as _time

                _time.sleep(2.0 * (attempt + 1))
        results.append(res[0])

    LAST_RUN["exec_time_ns"] = None

    out = np.zeros((T_all, O), np.float32)
    for c in range(len(ncs)):
        out[tok_lists[c]] = results[c]["yT"].T.astype(np.float32)
    return out.reshape(B, S, O)


# revision 9
# speedup vs baseline: 1.0022x; 1.0022x over previous
"""Ribbon MoE kernel: data-parallel over tokens with chain-ordered classes.

Each core gets ~2048 whole tokens (x loaded ONCE, y written ONCE — the
baseline expert-parallel kernel paid 2x on both). Tokens are ordered along
an Eulerian circuit of the expert-pair multigraph so that every expert's
tokens form contiguous ranges; each core's arc is a chain of class pieces.

Per-core program (compiled per core — ranges are core-specific constants):
  mm1  h[r, t] (PSUM) = sum_kc Wa[slot]^T @ xT       per expert occurrence
  scale h_sb = h * combine-weight (broadcast via 1-partition ones matmul)
  mm2  yT[o, t] (PSUM) accumulates BOTH experts of each token, per o-chunk
  out  yT -> DRAM bf16; host transposes and scatters.
"""

import os
import sys
from contextlib import ExitStack

import numpy as np

if os.environ.get("JAX_PLATFORMS", None) == "cpu" and "jax" not in sys.modules:
    os.environ.pop("JAX_PLATFORMS")

for _p in ("/opt/trn_rl_repo",):
    if _p not in sys.path and os.path.isdir(_p):
        sys.path.insert(0, _p)

import ml_dtypes  # noqa: E402

import concourse.tile as tile  # noqa: E402
from concourse import bacc, mybir  # noqa: E402

BF16 = mybir.dt.bfloat16
NP_BF16 = ml_dtypes.bfloat16
F32 = mybir.dt.float32

N_EXPERTS = 8
D = 2048
R = 128
O = 2048
KC = D // 128
TB = 512

_PROGRAM_CACHE: dict[tuple, object] = {}
LAST_RUN = {"exec_time_ns": None, "mean_exec_time_ns": None, "per_core_ns": None}


# ---------------------------------------------------------------- host: ribbon
def _route(x, router_w):
    norm = np.maximum(np.sqrt(np.einsum("td,td->t", x, x, dtype=np.float64)), 1e-12)
    logits = (x @ router_w) / norm[:, None].astype(np.float32)
    m = logits.max(-1, keepdims=True)
    p = np.exp(logits - m, dtype=np.float32)
    p /= p.sum(-1, keepdims=True)
    t_idx = np.arange(x.shape[0])
    e1 = p.argmax(-1)
    w1 = p[t_idx, e1]
    p2 = p.copy()
    p2[t_idx, e1] = -np.inf
    e2 = p2.argmax(-1)
    w2 = p[t_idx, e2]
    s = w1 + w2
    return e1, e2, (w1 / s).astype(np.float32), (w2 / s).astype(np.float32)


def _eulerian_circuit(edges, rng):
    adj = {}
    for i, (a, b) in enumerate(edges):
        adj.setdefault(a, []).append((b, i))
        adj.setdefault(b, []).append((a, i))
    if rng is not None:
        for v in adj:
            rng.shuffle(adj[v])
    used = [False] * len(edges)
    stack = [edges[0][0]]
    path = []
    while stack:
        v = stack[-1]
        found = False
        while adj[v]:
            u, ei = adj[v].pop()
            if not used[ei]:
                used[ei] = True
                stack.append(u)
                found = True
                break
        if not found:
            path.append(stack.pop())
    if not all(used):
        raise RuntimeError("not connected")
    return path[::-1]


_MATCHINGS = [
    [(0, 1), (2, 3), (4, 5), (6, 7)],
    [(0, 2), (1, 3), (4, 6), (5, 7)],
    [(0, 3), (1, 2), (4, 7), (5, 6)],
    [(0, 4), (1, 5), (2, 6), (3, 7)],
    [(0, 5), (1, 4), (2, 7), (3, 6)],
    [(0, 6), (1, 7), (2, 4), (3, 5)],
    [(0, 7), (1, 6), (2, 5), (3, 4)],
]


def _build_circuit(class_tokens, matching, seed):
    edges = sorted(class_tokens.keys())
    dup = [tuple(sorted(m)) for m in matching if tuple(sorted(m)) in class_tokens]
    deg = np.zeros(N_EXPERTS, int)
    for a, b in edges + dup:
        deg[a] += 1
        deg[b] += 1
    odd = [v for v in range(N_EXPERTS) if deg[v] % 2]
    while odd:
        a = odd.pop()
        for b in odd:
            if (min(a, b), max(a, b)) in class_tokens:
                dup.append((min(a, b), max(a, b)))
                odd.remove(b)
                break
        else:
            raise RuntimeError("cannot fix parity")
    rng = np.random.default_rng(seed)
    circuit = _eulerian_circuit(edges + dup, rng)
    inst = [(circuit[i], circuit[i + 1]) for i in range(len(circuit) - 1)]
    n_inst_of = {}
    for va, vb in inst:
        key = (min(va, vb), max(va, vb))
        n_inst_of[key] = n_inst_of.get(key, 0) + 1
    inst_counts = []
    seen = {}
    for va, vb in inst:
        key = (min(va, vb), max(va, vb))
        k, n = n_inst_of[key], class_tokens[key].size
        j = seen.get(key, 0)
        seen[key] = j + 1
        inst_counts.append(n // k + (1 if j < n % k else 0))
    return inst, inst_counts


def _cut_arcs(inst, inst_counts, cuts):
    bounds = np.concatenate([[0], np.cumsum(inst_counts)])
    arcs = []
    for c in range(len(cuts) - 1):
        lo, hi = cuts[c], cuts[c + 1]
        pieces = []
        for i, (va, vb) in enumerate(inst):
            a, b = max(bounds[i], lo), min(bounds[i + 1], hi)
            if a < b:
                pieces.append((va, vb, int(b - a)))
        arcs.append(pieces)
    return arcs


def _arc_cost(pieces):
    T = sum(p[2] for p in pieces)
    verts = [pieces[0][0]] + [p[1] for p in pieces]
    ns = len(set(verts))
    dma = (T * D * 2 * 2 + ns * 2 * 128 * 2048 * 2 + T * 2 * 2 * 2) / 360.0
    pe = (2 * T * 32 + 2 * T) / 2.4
    return max(dma, pe) + 3500.0


def _balance_cuts(inst, inst_counts, n_cores=8, iters=200):
    T = int(np.sum(inst_counts))
    cuts = [round(T * i / n_cores) for i in range(n_cores + 1)]
    best = None
    for _ in range(iters):
        arcs = _cut_arcs(inst, inst_counts, cuts)
        costs = [_arc_cost(a) for a in arcs]
        worst = max(costs)
        if best is None or worst < best[0]:
            best = (worst, list(cuts))
        w = int(np.argmax(costs))
        moved = False
        for delta in (64, 32, 16, 8):
            if w > 0 and costs[w - 1] < worst:
                nc_ = list(cuts)
                nc_[w] += delta
                if nc_[w] < nc_[w + 1]:
                    cuts, moved = nc_, True
                    break
            if w < n_cores - 1 and costs[w + 1] < worst:
                nc_ = list(cuts)
                nc_[w + 1] -= delta
                if nc_[w + 1] > nc_[w]:
                    cuts, moved = nc_, True
                    break
        if not moved:
            break
    return best[1]


def _plan_ribbon(e1, e2, n_cores=8, n_trials=120):
    """-> (class_tokens, inst, inst_counts, cuts) minimizing max core cost."""
    lo = np.minimum(e1, e2)
    hi = np.maximum(e1, e2)
    cid = lo * N_EXPERTS + hi
    class_tokens = {}
    for a in range(N_EXPERTS):
        for b in range(a + 1, N_EXPERTS):
            idx = np.nonzero(cid == a * N_EXPERTS + b)[0]
            if idx.size:
                class_tokens[(a, b)] = idx
    best = None
    for trial in range(n_trials):
        matching = _MATCHINGS[trial % len(_MATCHINGS)]
        try:
            inst0, counts0 = _build_circuit(class_tokens, matching, trial)
        except RuntimeError:
            continue
        # the circuit is closed, so any rotation of the instance list is
        # also a valid chain ordering; rotations move the cut boundaries
        # relative to the classes (different NS distributions)
        for rot in range(0, len(inst0), 4):
            inst = inst0[rot:] + inst0[:rot]
            inst_counts = counts0[rot:] + counts0[:rot]
            cuts = _balance_cuts(inst, inst_counts, n_cores)
            arcs = _cut_arcs(inst, inst_counts, cuts)
            cost = max(_arc_cost(a) for a in arcs)
            if best is None or cost < best[0]:
                best = (cost, inst, inst_counts, cuts)
    _, inst, inst_counts, cuts = best
    return class_tokens, inst, inst_counts, cuts


# ------------------------------------------------------------- device program
def _arc_meta(pieces):
    """Static per-core structure used both for program build and host arrays.

    Returns dict with:
      T, K, bounds b[0..K], verts v[0..K], slots (distinct experts in first-
      appearance order), slot_of[j] for each occurrence j, occ ranges
      (s_j, e_j), h offsets o_j, H.
    """
    K = len(pieces)
    lens = [p[2] for p in pieces]
    bounds = [0]
    for L in lens:
        bounds.append(bounds[-1] + L)
    T = bounds[-1]
    verts = [pieces[0][0]] + [p[1] for p in pieces]
    slots = []
    slot_of = []
    for v in verts:
        if v not in slots:
            slots.append(v)
        slot_of.append(slots.index(v))
    occ = []
    for j in range(K + 1):
        s = bounds[max(j - 1, 0)]
        e = bounds[min(j + 1, K)]
        occ.append((s, e))
    offs = [0]
    for s, e in occ:
        offs.append(offs[-1] + (e - s))
    H = offs[-1]
    return dict(
        T=T, K=K, bounds=bounds, verts=verts, slots=slots, slot_of=slot_of,
        occ=occ, offs=offs[:-1], H=H, NS=len(slots), variant=0,
    )


def _prog_key(meta):
    return (
        meta["T"], tuple(meta["bounds"]), tuple(meta["slot_of"]), meta["NS"],
        meta.get("variant", 0),
    )


def _build_program(meta):
    T, K, bounds = meta["T"], meta["K"], meta["bounds"]
    occ, offs, H, NS = meta["occ"], meta["offs"], meta["H"], meta["NS"]
    slot_of = meta["slot_of"]
    variant = meta.get("variant", 0)
    v_wb0_first = bool(variant & 1)  # need-0 wb slots before x1 in the head
    v_mm20_first = not bool(variant & 2)  # mm2(0) before mm1(1)
    v_two_small = bool(variant & 4)  # two small head blocks
    v_small_tail2 = bool(variant & 8)  # two small tail blocks
    n_warm = 6 if variant & 16 else 10  # PE p-state warmup matmuls
    v_x0b_first = bool(variant & 32)  # x0b before wa0b/cv in the head

    nc = bacc.Bacc("TRN2", target_bir_lowering=False, debug=False, num_devices=1)
    xT = nc.dram_tensor("xT", [D, T], BF16, kind="ExternalInput").ap()
    wa = nc.dram_tensor("wa", [NS, 128, KC * R], BF16, kind="ExternalInput").ap()
    wb = nc.dram_tensor("wb", [NS, 128, O], BF16, kind="ExternalInput").ap()
    cv = nc.dram_tensor("cv", [1, H], BF16, kind="ExternalInput").ap()
    yT = nc.dram_tensor("yT", [O, T], BF16, kind="ExternalOutput").ap()

    xTr = xT.rearrange("(kc p) t -> p kc t", p=128)
    yTr = yT.rearrange("(g p) t -> p g t", p=128)
    war = wa.rearrange("s p f -> p s f")
    wbr = wb.rearrange("s p f -> p s f")
    # block sizes: small head (fast pipeline prime) and tail (fast drain),
    # balanced middle; every size ≥ 256 tokens keeps DMA lines ≥ 512B.
    if T <= 1024:
        n_blk = (T + TB - 1) // TB
        bsz = [T // n_blk + (1 if i < T % n_blk else 0) for i in range(n_blk)]
    elif v_two_small and v_small_tail2 and T > 1536:
        mid = T - 1024
        nmid = (mid + TB - 1) // TB
        bsz = (
            [256, 256]
            + [mid // nmid + (1 if i < mid % nmid else 0) for i in range(nmid)]
            + [256, 256]
        )
    elif v_two_small and T > 1280:
        mid = T - 768
        nmid = (mid + TB - 1) // TB
        bsz = (
            [256, 256]
            + [mid // nmid + (1 if i < mid % nmid else 0) for i in range(nmid)]
            + [256]
        )
    elif v_small_tail2 and T > 1280:
        mid = T - 768
        nmid = (mid + TB - 1) // TB
        bsz = (
            [256]
            + [mid // nmid + (1 if i < mid % nmid else 0) for i in range(nmid)]
            + [256, 256]
        )
    else:
        mid = T - 512
        nmid = (mid + TB - 1) // TB
        bsz = (
            [256]
            + [mid // nmid + (1 if i < mid % nmid else 0) for i in range(nmid)]
            + [256]
        )
    n_blk = len(bsz)
    bstart = [0]
    for s in bsz:
        bstart.append(bstart[-1] + s)

    def blk(b):
        return bstart[b], bstart[b + 1]

    # first block needing each weight slot (slots are numbered in chain
    # first-appearance order, so first use of slot s is occ j with
    # slot_of[j] == s, covering tokens from occ[j][0])
    first_tok = {}
    for j in range(K + 1):
        s = slot_of[j]
        if s not in first_tok:
            first_tok[s] = occ[j][0]

    def tok_to_blk(t):
        for b in range(n_blk):
            if t < bstart[b + 1]:
                return b
        return n_blk - 1

    need_blk = {s: tok_to_blk(first_tok[s]) for s in range(NS)}

    with tile.TileContext(nc) as tc, ExitStack() as ctx:
        wpool = ctx.enter_context(tc.tile_pool(name="w", bufs=1))
        xpool = ctx.enter_context(tc.tile_pool(name="x", bufs=3))
        hpool = ctx.enter_context(tc.tile_pool(name="h", bufs=1))
        ypool = ctx.enter_context(tc.tile_pool(name="y", bufs=6))
        hps = ctx.enter_context(tc.tile_pool(name="hps", bufs=2, space="PSUM"))
        yps = ctx.enter_context(tc.tile_pool(name="yps", bufs=3, space="PSUM"))

        # ---- head: cv (tiny) first, then wa slot 0 + x block 0 so mm1(0)
        # starts early; remaining weight slots are loaded just-in-time in
        # chain order, interleaved with x blocks. While the head DMAs land,
        # PE broadcasts the combine weights (dummy+cb matmuls double as
        # p-state warmup); Pool does the PSUM->SBUF cb copies (idle early).
        ones_sb = wpool.tile([1, TB], BF16)
        nc.gpsimd.memset(ones_sb[:], 1.0)
        cv_sb = wpool.tile([1, H], BF16)
        wa_sb = wpool.tile([128, NS, KC * R], BF16)
        wb_sb = wpool.tile([128, NS, O], BF16)
        cb_sb = wpool.tile([128, H], BF16)
        loaded = set()
        loaded_b = set()

        def load_wa(s):
            if s in loaded:
                return
            loaded.add(s)
            nc.sync.dma_start(wa_sb[:, s, :], war[:, s, :])

        def load_wb(s):
            if s in loaded_b:
                return
            loaded_b.add(s)
            nc.sync.dma_start(wb_sb[:, s, :], wbr[:, s, :])

        xts = {}
        def load_x_half(b, half):
            t0, t1 = blk(b)
            tb = t1 - t0
            if half == 0:
                xt = xpool.tile([128, KC, TB], BF16, tag="xt")
                xts[b] = xt
            q = KC // 2
            nc.sync.dma_start(
                xts[b][:, half * q : (half + 1) * q, :tb],
                xTr[:, half * q : (half + 1) * q, t0:t1],
            )

        def jit_wa(lim):
            for s in range(NS):
                if need_blk[s] <= lim:
                    load_wa(s)

        def jit_wb(lim):
            for s in range(NS):
                if need_blk[s] <= lim:
                    load_wb(s)

        nc.sync.dma_start(wa_sb[:, 0, : KC * R // 2], war[:, 0, : KC * R // 2])
        loaded.add(0)
        load_x_half(0, 0)
        if v_x0b_first:
            load_x_half(0, 1)
            nc.sync.dma_start(
                wa_sb[:, 0, KC * R // 2 :], war[:, 0, KC * R // 2 :]
            )
            nc.sync.dma_start(cv_sb[:], cv[:])
        else:
            nc.sync.dma_start(cv_sb[:], cv[:])
            nc.sync.dma_start(
                wa_sb[:, 0, KC * R // 2 :], war[:, 0, KC * R // 2 :]
            )
            load_x_half(0, 1)
        jit_wa(1)
        head_wb = (
            [s for s in range(NS) if need_blk[s] == 0] if v_wb0_first else [0]
        )
        for s in head_wb:  # first halves: unblock mm2(0) o-chunks 0-7
            loaded_b.add(s)
            nc.sync.dma_start(wb_sb[:, s, : O // 2], wbr[:, s, : O // 2])
        if n_blk > 1:
            load_x_half(1, 0)
        for s in head_wb:
            nc.sync.dma_start(wb_sb[:, s, O // 2 :], wbr[:, s, O // 2 :])
        if n_blk > 1:
            load_x_half(1, 1)
        jit_wb(1)

        # PE warmup: dummy matmuls on the memset ones tile (no DMA dep) keep
        # PE continuously busy from ~0.4us so the p-state ramp completes by
        # the time mm1(0)'s inputs land.
        for _ in range(n_warm):
            warm_ps = yps.tile([128, 2, TB], F32, tag="yp")
            nc.tensor.matmul(
                warm_ps[:, 0, :TB], ones_sb[:, :128], ones_sb[:, :],
                start=True, stop=True,
            )
        # broadcast combine weights to 128 partitions on Pool (idle early,
        # SBUF->SBUF): first the columns blocks 0-1 read, then the rest.
        lim = bstart[min(2, n_blk)]
        cb_head = max(
            offs[j] + (min(occ[j][1], lim) - occ[j][0])
            for j in range(K + 1)
            if occ[j][0] < lim
        )
        nc.gpsimd.partition_broadcast(
            cb_sb[:, :cb_head], cv_sb[0:1, :cb_head]
        )
        if cb_head < H:
            nc.gpsimd.partition_broadcast(
                cb_sb[:, cb_head:], cv_sb[0:1, cb_head:]
            )

        h_sb = hpool.tile([128, H], BF16)

        def mm1_block(b):
            t0, t1 = blk(b)
            for j in range(K + 1):
                s_j, e_j = occ[j]
                ss, se = max(s_j, t0), min(e_j, t1)
                if ss >= se:
                    continue
                seg = se - ss
                ho = offs[j] + (ss - s_j)
                hp = hps.tile([128, TB], F32, tag="hp")
                for kc in range(KC):
                    nc.tensor.matmul(
                        hp[:, :seg],
                        wa_sb[:, slot_of[j], kc * R : (kc + 1) * R],
                        xts[b][:, kc, ss - t0 : se - t0],
                        start=(kc == 0),
                        stop=(kc == KC - 1),
                    )
                nc.vector.tensor_tensor(
                    h_sb[:, ho : ho + seg],
                    hp[:, :seg],
                    cb_sb[:, ho : ho + seg],
                    mybir.AluOpType.mult,
                )

        copy_engines = [nc.vector, nc.scalar, nc.gpsimd]

        deferred_y = []

        def flush_y(upto_b):
            # early-block y stores issue from the DVE queue, positioned after
            # later blocks' h-scales, so their DMA requests don't compete
            # with x/weight loads during the input-heavy phase
            while deferred_y and deferred_y[0][0] <= upto_b:
                _, ys_t, hf_, t0_, t1_, tb_ = deferred_y.pop(0)
                nc.scalar.dma_start(
                    yTr[:, hf_ * 8 : (hf_ + 1) * 8, t0_:t1_], ys_t[:, :, :tb_]
                )

        def mm2_block(b):
            t0, t1 = blk(b)
            tb = t1 - t0
            for hf in range(2):  # 8 o-chunks per half; separate y store each
                ys = ypool.tile([128, 8, TB], BF16, tag="ys")
                for pair in range(4):
                    yp = yps.tile([128, 2, TB], F32, tag="yp")
                    for u in range(2):
                        oc = hf * 8 + pair * 2 + u
                        for i in range(K):
                            ps = max(bounds[i], t0)
                            pe = min(bounds[i + 1], t1)
                            if ps >= pe:
                                continue
                            for j, start in ((i, True), (i + 1, False)):
                                s_j, _ = occ[j]
                                ho = offs[j] + (ps - s_j)
                                nc.tensor.matmul(
                                    yp[:, u, ps - t0 : pe - t0],
                                    wb_sb[:, slot_of[j], oc * 128 : (oc + 1) * 128],
                                    h_sb[:, ho : ho + (pe - ps)],
                                    start=start,
                                    stop=not start,
                                )
                    dst = ys[:, pair * 2 : pair * 2 + 2, :tb]
                    if (hf * 4 + pair) % 2 == 0:
                        nc.vector.tensor_copy(dst, yp[:, :, :tb])
                    else:
                        nc.scalar.copy(dst, yp[:, :, :tb])
                    if b == n_blk - 1:
                        # drain: store each pair via HWDGE as soon as its
                        # copy lands (no SWDGE fixed cost, short tail chain)
                        oc0 = hf * 8 + pair * 2
                        nc.sync.dma_start(
                            yTr[:, oc0 : oc0 + 2, t0:t1],
                            ys[:, pair * 2 : pair * 2 + 2, :tb],
                        )
                if b < n_blk - 3:
                    deferred_y.append((b, ys, hf, t0, t1, tb))
                elif b != n_blk - 1:
                    nc.gpsimd.dma_start(
                        yTr[:, hf * 8 : (hf + 1) * 8, t0:t1], ys[:, :, :tb]
                    )

        # software pipeline: x loads 2 blocks ahead; mm1(b+1) before mm2(b)
        # software pipeline; block 0 computes mm2 before mm1(1) (x1 lands
        # late during the head burst, and mm2(0)'s inputs are ready first)
        mm1_block(0)
        for b in range(n_blk):
            if b + 2 < n_blk:
                load_x_half(b + 2, 0)
                jit_wa(b + 3)
                load_x_half(b + 2, 1)
                jit_wb(b + 2)
            if b == 0 and v_mm20_first:
                mm2_block(0)
                if n_blk > 1:
                    mm1_block(1)
                continue
            if b + 1 < n_blk:
                mm1_block(b + 1)
            flush_y(b - 2)
            mm2_block(b)
        flush_y(n_blk)
        assert not deferred_y
        assert len(loaded) == NS and len(loaded_b) == NS, (loaded, loaded_b, need_blk)

    nc.compile()
    return nc


def _get_program(meta):
    key = _prog_key(meta)
    if key not in _PROGRAM_CACHE:
        _PROGRAM_CACHE[key] = _build_program(meta)
    return _PROGRAM_CACHE[key]


def _sim_ns(meta):
    from concourse.timeline_sim import TimelineSim

    key = ("sim",) + _prog_key(meta)
    if key not in _PROGRAM_CACHE:
        _PROGRAM_CACHE[key] = int(TimelineSim(_get_program(meta)).simulate())
    return _PROGRAM_CACHE[key]


_VARIANT_SET = [0, 1, 3, 4, 5, 7, 8, 12, 16, 17, 20, 21, 23, 28]


def _pick_variants(arcs, variants=None):
    """Per-core schedule variant chosen by TimelineSim (per-core programs
    make heterogeneous schedules free)."""
    if variants is None:
        variants = _VARIANT_SET
    metas = []
    for pieces in arcs:
        best = None
        for v in variants:
            m = _arc_meta(pieces)
            m["variant"] = v
            t = _sim_ns(m)
            if best is None or t < best[0]:
                best = (t, m)
        metas.append(best[1])
    return metas


def _refine_cuts(inst, inst_counts, cuts, iters=6, tol_ns=400):
    """Move cut positions between adjacent cores based on actual TimelineSim
    costs of the compiled per-core programs (the static cost model misses
    schedule overheads that differ per core)."""
    T = int(np.sum(inst_counts))
    n_cores = len(cuts) - 1
    best_cuts, best_max = list(cuts), None
    for _ in range(iters):
        arcs = _cut_arcs(inst, inst_counts, cuts)
        times = [_sim_ns(m) for m in _pick_variants(arcs)]
        worst = max(times)
        if best_max is None or worst < best_max:
            best_max, best_cuts = worst, list(cuts)
        order = sorted(range(n_cores), key=lambda i: -times[i])
        moved = False
        for w in order[:3]:
            if times[w] < worst - tol_ns:
                break
            # shift tokens from w toward its cheaper neighbor
            for nb, edge in ((w - 1, w), (w + 1, w + 1)):
                if 0 <= nb < n_cores and times[nb] < times[w] - tol_ns:
                    delta = min(96, max(16, (times[w] - times[nb]) // 90))
                    nc_ = list(cuts)
                    if nb < w:
                        nc_[edge] += delta
                    else:
                        nc_[edge] -= delta
                    if nc_[edge] - nc_[edge - 1] >= 256 and nc_[edge + 1] - nc_[edge] >= 256:
                        cuts, moved = nc_, True
                        break
            if moved:
                break
        if not moved:
            break
    return best_cuts


# ------------------------------------------------------------------ execution
def kernel(hidden_states, router_w, Wa, Wb):
    B, S, _ = hidden_states.shape
    x = np.ascontiguousarray(
        np.asarray(hidden_states, dtype=np.float32).reshape(-1, D)
    )
    T_all = x.shape[0]
    router_w = np.asarray(router_w, dtype=np.float32)
    Wa = np.asarray(Wa, dtype=np.float32)
    Wb = np.asarray(Wb, dtype=np.float32)

    e1, e2, c1, c2 = _route(x, router_w)
    cmat = np.zeros((T_all, N_EXPERTS), np.float32)
    cmat[np.arange(T_all), e1] = c1
    cmat[np.arange(T_all), e2] = c2

    class_tokens, inst, inst_counts, cuts0 = _plan_ribbon(e1, e2)
    best = None
    for it, tol in ((16, 300), (12, 150)):
        c2 = _refine_cuts(inst, inst_counts, cuts0, iters=it, tol_ns=tol)
        t2 = max(
            _sim_ns(m) for m in _pick_variants(_cut_arcs(inst, inst_counts, c2))
        )
        if best is None or t2 < best[0]:
            best = (t2, c2)
    cuts = best[1]
    arcs = _cut_arcs(inst, inst_counts, cuts)

    # global token order: per class, tokens consumed across its instances
    consumed = {k: 0 for k in class_tokens}
    order_parts = []
    for (va, vb), cnt in zip(inst, inst_counts):
        key = (min(va, vb), max(va, vb))
        s = consumed[key]
        order_parts.append(class_tokens[key][s : s + cnt])
        consumed[key] = s + cnt
    order = np.concatenate(order_parts)
    assert order.size == T_all

    x_bf = x.astype(NP_BF16)

    metas0 = _pick_variants(arcs)
    metas, in_maps, tok_lists = [], [], []
    for c, pieces in enumerate(arcs):
        meta = metas0[c]
        toks = order[cuts[c] : cuts[c + 1]]
        assert toks.size == meta["T"]
        xTc = np.ascontiguousarray(x_bf[toks].T)
        wa_arr = np.stack(
            [
                np.ascontiguousarray(
                    Wa[v].reshape(KC, 128, R).transpose(1, 0, 2).reshape(128, KC * R)
                ).astype(NP_BF16)
                for v in meta["slots"]
            ]
        )
        wb_arr = np.stack([Wb[v].astype(NP_BF16) for v in meta["slots"]])
        cv_arr = np.zeros((1, meta["H"]), np.float32)
        for j in range(meta["K"] + 1):
            s_j, e_j = meta["occ"][j]
            seg_toks = toks[s_j:e_j]
            cv_arr[0, meta["offs"][j] : meta["offs"][j] + (e_j - s_j)] = cmat[
                seg_toks, meta["verts"][j]
            ]
        metas.append(meta)
        tok_lists.append(toks)
        in_maps.append(
            {"xT": xTc, "wa": wa_arr, "wb": wb_arr, "cv": cv_arr.astype(NP_BF16)}
        )

    ncs = [_get_program(m) for m in metas]
    LAST_RUN["metas"] = metas

    import jax
    from concourse import bass2jax

    devices = jax.devices()
    results = []
    for c in range(len(ncs)):
        for attempt in range(3):
            try:
                with jax.default_device(devices[c % len(devices)]):
                    res = bass2jax.run_bass_via_pjrt(
                        ncs[c], [in_maps[c]], n_cores=1
                    )
                break
            except Exception:  # transient NRT_EXEC_UNIT_UNRECOVERABLE etc.
                if attempt == 2:
                    raise
                try:
                    import jax.extend.backend

                    jax.extend.backend.clear_backends()
                except Exception:
                    pass
                import time as _time

                _time.sleep(2.0 * (attempt + 1))
        results.append(res[0])

    LAST_RUN["exec_time_ns"] = None

    out = np.zeros((T_all, O), np.float32)
    for c in range(len(ncs)):
        out[tok_lists[c]] = results[c]["yT"].T.astype(np.float32)
    return out.reshape(B, S, O)
